# revision 33
# baseline (speedup 1.0000x reference)
"""Trainium2 Bass kernel for nn_AttnBlock (B=8, C=64, H=W=64).

Data-parallel: 1 batch per NeuronCore (8 cores). Per core, full
flash-style attention over N=4096 positions with C=64 channels,
never materializing the (N, N) score tensor in HBM.

Per-core pipeline (all on-chip, x kept resident in SBUF):
  1. 8-way X DMA; GroupNorm via bn_stats + pair-combine matmul and a
     Quake-style rsqrt on the vector engine (avoids the sqrt act-table
     load; only the exp table set is ever loaded, via an early dummy).
  2. Transpose xn (c, n) -> XT (n, c) bf16 tiles via PE transposes;
     scalar-engine copies drain the transpose PSUM.
  3. Projections q/k (bf16, both array halves filled by an SBUF->SBUF
     dup DMA) and v in fp8e4 (scaled x64, folded back via Wp) with a
     ones column so row sums of exp(S) fall out of the AV matmul.
  4. Attention: S^T tiles via bf16 matmuls alternating PE halves.
     exp() is split across TWO engines: scalar-engine Exp -> fp8e4 and
     vector-engine Schraudolph bit-trick (s*a+b -> uint8, bitcast
     fp8e4), both writing a 12-slot fp8 P ring. AV matmuls run in fp8
     DoubleRow mode over m-tile pairs (2 contractions/cycle).
  5. Normalize/project/residual epilogue of chunk i interleaved into
     chunk i+1; PE pre-warmed with dummy matmuls so the HAM clock gate
     reaches 2.4 GHz before real work.

Self-contained: hardcodes all shapes; no file reads.
"""

import numpy as np
from contextlib import ExitStack

import concourse.bass as bass
import concourse.bacc as bacc
import concourse.tile as tile
from concourse import mybir

F32 = mybir.dt.float32
BF16 = mybir.dt.bfloat16
FP8 = mybir.dt.float8e4
U8 = mybir.dt.uint8
I16 = mybir.dt.int16
I32 = mybir.dt.int32

C = 64
N = 4096          # H*W
NCH = 8           # n-chunks of 512
CHW = 512         # chunk width
MT = 32           # m-tiles of 128
RING = 12         # P-ring slots
EPS = 1e-5
SCALE = 1.0 / 8.0  # C ** -0.5
A16 = 128.0 * 1.4426950408889634 * SCALE  # Schraudolph slope for bf16
B16 = 16128.0    # 127*128 exponent bias, shifted -ln2 (matches ACT bias)
QMAGIC = 0x5F3759DF

GSZ = 2
GROUPS = [(m0, min(GSZ, MT - m0)) for m0 in range(0, MT, GSZ)]

Exp = mybir.ActivationFunctionType.Exp
Copy = mybir.ActivationFunctionType.Copy
mult = mybir.AluOpType.mult
add = mybir.AluOpType.add
sub = mybir.AluOpType.subtract
b_xor = mybir.AluOpType.bitwise_xor
shr = mybir.AluOpType.arith_shift_right


def attn_body(ctx: ExitStack, tc: "tile.TileContext", ins: dict, y_d):
    nc = tc.nc

    persist = ctx.enter_context(tc.tile_pool(name="persist", bufs=1))
    sm = ctx.enter_context(tc.tile_pool(name="sm", bufs=2))

    # ---- persistent SBUF tiles ----
    X = persist.tile([C, N], F32, tag="X")
    XN = persist.tile([C, N], BF16, tag="XN")
    XT = persist.tile([128, 2048], BF16, tag="XT")      # ((h,w), c) tiles
    Q = persist.tile([128, N], BF16, tag="Q")
    K = persist.tile([128, N], BF16, tag="K")
    VT1 = persist.tile([128, MT, 65], BF16, tag="VT1")  # ((H,j), c | 1)
    PT = persist.tile([128, RING * CHW], BF16, tag="PT")  # exp(S^T) ring
    OUT = persist.tile([C, N], F32, tag="OUT")
    WS = persist.tile([128, CHW], BF16, tag="WS")       # warmup scratch

    PF = persist.tile([128, 1220], F32, tag="PF")
    PB = persist.tile([128, 640], BF16, tag="PB")
    ZB = persist.tile([128, 1], F32, tag="ZB")
    NLN2 = persist.tile([128, 1], F32, tag="NLN2")
    ID = PF[:, 0:128]          # fp32 identity (epilogue transposes)
    BQ2D = PF[:, 128:640]      # q bias along free axis, tiled x4
    BK2D = PF[:, 640:1152]
    P2 = PF[0:C, 1152:1216]    # pair-combine matrix
    GM = PF[0:C, 1216:1217]
    BT = PF[0:C, 1217:1218]
    BV2S = PF[:, 1218:1219]    # tile(bv,2)
    BP2 = PF[:, 1219:1220]
    PRJ = PB[:, 0:256]         # [Wq-blockdiag | Wk-blockdiag]
    WV = PB[:, 256:384]
    WPB = PB[:, 384:512]       # blockdiag(Wp)/VSCALE
    IDB = PB[:, 512:640]       # bf16 identity

    # ---- memsets + act-table prime + PE warmup + DMAs ----
    nc.vector.memset(ZB, 0.0)
    nc.vector.memset(NLN2, -0.6931471805599453)
    nc.vector.memset(WS, 0.0)
    nc.vector.memset(VT1[:, :, 64:65], 1.0)  # ones column -> row sums
    # dummy exp: pins the exp_and_others table load into setup dead time
    dume = sm.tile([C, 1], F32, tag="dume")
    nc.scalar.activation(out=dume, in_=ZB[0:C, :], func=Exp,
                         bias=ZB[0:C, :], scale=1.0)

    # X in 8 slices across the DMA-capable queues; params after
    engs = (nc.sync, nc.scalar, nc.gpsimd)
    for i in range(8):
        engs[i % 3].dma_start(out=X[:, i * 512:(i + 1) * 512],
                              in_=ins["x"][:, i * 512:(i + 1) * 512])
    nc.sync.dma_start(out=PF, in_=ins["pf32"])
    nc.scalar.dma_start(out=PB, in_=ins["pb16"])

    with tc.tile_pool(name="pst", space="PSUM", bufs=3) as pst, \
         tc.tile_pool(name="warm", space="PSUM", bufs=1) as warm:
        # ---- HAM warmup: dummy matmuls gated on the last X slice so the
        # burst lands right before the block transposes (warm at ~22us) ----
        wps = warm.tile([128, CHW], F32, tag="wps")
        for _ in range(6):
            nc.tensor.matmul(wps, lhsT=WS[:, 0:128], rhs=WS,
                             start=True, stop=True)
        for _ in range(7):  # fp32 matmuls: 4x cycles each, ~5us of PE busy
            nc.tensor.matmul(wps[0:128, :], lhsT=X[:, 0:128],
                             rhs=X[:, 3584:4096], start=True, stop=True)

        # ---- GroupNorm stats (chasing the DMAs) ----
        stats = sm.tile([C, 8, 6], F32, tag="stats")
        xg = X.rearrange("p (s f) -> p s f", s=8)
        for s in range(8):
            nc.vector.bn_stats(out=stats[:, s, :], in_=xg[:, s, :])
        mv = sm.tile([C, 2], F32, tag="mv")
        nc.vector.bn_aggr(out=mv, in_=stats)

        # st = [mean, E[x^2]] per channel
        st = sm.tile([C, 2], F32, tag="st")
        nc.vector.tensor_copy(out=st[:, 0:1], in_=mv[:, 0:1])
        msq = sm.tile([C, 1], F32, tag="msq")
        nc.vector.tensor_tensor(out=msq, in0=mv[:, 0:1], in1=mv[:, 0:1], op=mult)
        nc.vector.tensor_tensor(out=st[:, 1:2], in0=msq, in1=mv[:, 1:2], op=add)

        # group (pair) averages, broadcast back to both partitions
        mg_ps = pst.tile([128, 1024], F32, tag="pp")
        nc.tensor.matmul(mg_ps[0:C, 0:2], lhsT=P2, rhs=st, start=True, stop=True)

        mu = sm.tile([C, 1], F32, tag="mu")
        nc.vector.tensor_copy(out=mu, in_=mg_ps[0:C, 0:1])
        musq = sm.tile([C, 1], F32, tag="musq")
        nc.vector.tensor_tensor(out=musq, in0=mu, in1=mu, op=mult)
        ve = sm.tile([C, 1], F32, tag="ve")
        nc.vector.tensor_tensor(out=ve, in0=mg_ps[0:C, 1:2], in1=musq, op=sub)
        nc.vector.tensor_scalar_add(out=ve, in0=ve, scalar1=EPS)

        # Quake rsqrt + 2 Newton iterations (all [64,1] DVE ops)
        ti = sm.tile([C, 1], I32, tag="ti")
        nc.vector.tensor_scalar(out=ti, in0=ve.bitcast(I32), scalar1=1,
                                scalar2=None, op0=shr)
        nc.vector.tensor_scalar(out=ti, in0=ti, scalar1=-1,
                                scalar2=None, op0=b_xor)
        nc.vector.tensor_scalar(out=ti, in0=ti, scalar1=QMAGIC + 1,
                                scalar2=None, op0=add)
        y0 = ti.bitcast(F32)
        rstd = sm.tile([C, 1], F32, tag="rstd")
        t_a = sm.tile([C, 1], F32, tag="t_a")
        for it in range(1):
            yin = y0 if it == 0 else rstd
            nc.vector.tensor_tensor(out=t_a, in0=yin, in1=yin, op=mult)
            nc.vector.tensor_tensor(out=t_a, in0=t_a, in1=ve, op=mult)
            nc.vector.tensor_scalar(out=t_a, in0=t_a, scalar1=-0.5,
                                    scalar2=1.5, op0=mult, op1=add)
            nc.vector.tensor_tensor(out=rstd, in0=yin, in1=t_a, op=mult)

        sc = sm.tile([C, 1], F32, tag="sc")
        nc.vector.tensor_tensor(out=sc, in0=rstd, in1=GM, op=mult)
        t3 = sm.tile([C, 1], F32, tag="t3")
        nc.vector.tensor_tensor(out=t3, in0=mu, in1=sc, op=mult)
        sh = sm.tile([C, 1], F32, tag="sh")
        nc.vector.tensor_tensor(out=sh, in0=BT, in1=t3, op=sub)

        # ---- per 1024-col block: normalize, transpose, project ----
        # Transposes run one block ahead of projections so every PE op's
        # input is ready ~a block early: the PE stream stays contiguous
        # through the setup phase and the HAM clock-gate reaches 2.4 GHz.
        def emit_projv(blk):
            for g in range(2):
                qk4 = pst.tile([128, 1024], F32, tag="pp", name=f"qk{blk}{g}")
                for t in range(4):
                    i = blk * 8 + g * 4 + t
                    nc.tensor.matmul(qk4[0:C, t * 256:(t + 1) * 256],
                                     lhsT=XT[:, i * C:(i + 1) * C],
                                     rhs=PRJ, start=True, stop=True)
                qk4r = qk4.rearrange("p (a b) -> p a b", a=4)
                hsl = slice(blk * 1024 + g * 512, blk * 1024 + (g + 1) * 512)
                q_out = Q[0:C, hsl].rearrange("p (a b) -> p a b", a=4)
                k_out = K[0:C, hsl].rearrange("p (a b) -> p a b", a=4)
                bq4 = BQ2D[0:C].rearrange("p (a b) -> p a b", a=4)
                bk4 = BK2D[0:C].rearrange("p (a b) -> p a b", a=4)
                nc.vector.tensor_tensor(out=q_out, in0=qk4r[0:C, :, 0:128],
                                        in1=bq4, op=add)
                nc.vector.tensor_tensor(out=k_out, in0=qk4r[0:C, :, 128:256],
                                        in1=bk4, op=add)
            # v projection: one 512-wide matmul covers 8 tiles
            vp8 = pst.tile([128, 1024], F32, tag="pp", name=f"v{blk}")
            nc.tensor.matmul(vp8[:, 0:512], lhsT=WV,
                             rhs=XT[:, blk * 512:(blk + 1) * 512],
                             start=True, stop=True)
            nc.scalar.activation(
                out=VT1[:, blk * 8:(blk + 1) * 8, 0:C],
                in_=vp8[:, 0:512].rearrange("p (a b) -> p a b", a=8),
                func=mybir.ActivationFunctionType.Identity,
                bias=BV2S, scale=1.0)
            bsl = slice(blk * 1024, (blk + 1) * 1024)
            # replicate q/k into PE-array upper half via off-engine DMA
            nc.sync.dma_start(out=Q[C:128, bsl], in_=Q[0:C, bsl])
            nc.scalar.dma_start(out=K[C:128, bsl], in_=K[0:C, bsl])

        prevb = None
        for blk in range(4):
            bsl = slice(blk * 1024, (blk + 1) * 1024)
            nc.gpsimd.tensor_scalar(out=XN[:, bsl], in0=X[:, bsl],
                                    scalar1=sc, scalar2=sh, op0=mult, op1=add)
            # transpose 8 x (64,128) -> (128,64); 4 per psum tile
            for g in range(2):
                tp4 = pst.tile([128, 1024], BF16, tag="pp", name=f"tp{blk}{g}")
                for t in range(4):
                    i = blk * 8 + g * 4 + t
                    nc.tensor.transpose(out=tp4[:, t * 64:(t + 1) * 64],
                                        in_=XN[:, i * 128:(i + 1) * 128],
                                        identity=IDB[0:C, 0:C])
                i0 = blk * 8 + g * 4
                nc.scalar.activation(out=XT[:, i0 * 64:(i0 + 4) * 64],
                                     in_=tp4[:, 0:256], func=Copy,
                                     bias=0.0, scale=1.0)
            if prevb is not None:
                emit_projv(prevb)
            prevb = blk
        emit_projv(prevb)

    # ---- attention ----
    spool = ctx.enter_context(tc.tile_pool(name="spool", space="PSUM", bufs=3))
    opool = ctx.enter_context(tc.tile_pool(name="opool", space="PSUM", bufs=1))
    epool = ctx.enter_context(tc.tile_pool(name="epool", space="PSUM", bufs=1))
    esb = ctx.enter_context(tc.tile_pool(name="esb", bufs=2))

    osbs = {}

    def epilogue_steps(ch):
        """Normalize by 1/l, project through Wp, add bias+residual, DMA out.

        Three dense PE batches (transposes | Wp matmuls | back-transposes)
        spaced one group apart; the elementwise chain between them runs on
        ACT/DVE well before the next PE batch needs it, so the in-order PE
        queue never blocks on a cross-engine dependency."""
        osb = osbs.pop(ch)
        nsl = slice(ch * CHW, (ch + 1) * CHW)
        pat4 = epool.tile([128, 260], F32, tag="ep", name=f"pa{ch}")
        for t in range(4):
            nc.tensor.transpose(out=pat4[:, t * 65:(t + 1) * 65],
                                in_=osb[:, t * 128:(t + 1) * 128],
                                identity=ID[0:65, 0:65])
        yield
        rli4 = esb.tile([128, 4], F32, tag="rli")
        nc.vector.reciprocal(
            out=rli4.rearrange("p (a b) -> p a b", b=1),
            in_=pat4.rearrange("p (a b) -> p a b", a=4)[:, :, 64:65])
        atn4 = esb.tile([128, 256], BF16, tag="atn")
        for t in range(4):
            nc.scalar.activation(out=atn4[:, t * C:(t + 1) * C],
                                 in_=pat4[:, t * 65:t * 65 + C],
                                 func=Copy, bias=0.0, scale=rli4[:, t:t + 1])
        yield
        pp4 = epool.tile([128, 256], F32, tag="ep", name=f"pp{ch}")
        nc.tensor.matmul(pp4, lhsT=WPB, rhs=atn4, start=True, stop=True)
        yield
        otb4 = esb.tile([128, 256], BF16, tag="otb")
        nc.scalar.activation(out=otb4, in_=pp4,
                             func=mybir.ActivationFunctionType.Identity,
                             bias=BP2, scale=1.0)
        yield
        pf4 = epool.tile([C, 512], BF16, tag="ep", name=f"pf{ch}")
        for t in range(4):
            nc.tensor.transpose(out=pf4[:, t * 128:(t + 1) * 128],
                                in_=otb4[:, t * C:(t + 1) * C], identity=IDB)
        yield
        for h in range(2):
            csl = slice(ch * CHW + h * 256, ch * CHW + (h + 1) * 256)
            nc.vector.tensor_tensor(out=OUT[:, csl],
                                    in0=pf4[:, h * 256:(h + 1) * 256],
                                    in1=X[:, csl], op=add)
        nc.sync.dma_start(out=y_d[:, nsl], in_=OUT[:, nsl])

    pending = None
    av_q = []  # AV closures lag their exp by TWO groups: keeps the AV
               # matmuls (which wait on exp output) off the critical
               # exp -> S(psum-slot) -> exp dependency cycle

    def emit_sgroup(ps, m0, gsz, nsl):
        for t in range(gsz):
            m = m0 + t
            h = (m % 2) * C  # alternate the two 64-row PE halves
            nc.tensor.matmul(
                ps[:, t * CHW:(t + 1) * CHW],
                lhsT=K[h:h + C, m * 128:(m + 1) * 128],
                rhs=Q[h:h + C, nsl],
                start=True, stop=True, tile_position=(h, 0))

    hoisted_ps = None
    for ch in range(NCH):
        nsl = slice(ch * CHW, (ch + 1) * CHW)
        po = opool.tile([128, CHW], F32, tag="po")
        ctr = [0]  # per-chunk AV pair counter (closures capture their own)
        for gi, (m0, gsz) in enumerate(GROUPS):
            if gi == 0 and hoisted_ps is not None:
                ps = hoisted_ps
                hoisted_ps = None
            else:
                ps = spool.tile([128, GSZ * CHW], F32, tag="ps")
                emit_sgroup(ps, m0, gsz, nsl)
            if gi == len(GROUPS) - 1 and ch < NCH - 1:
                # hoist next chunk's first S-group ahead of trailing AV work
                hoisted_ps = spool.tile([128, GSZ * CHW], F32, tag="ps",
                                        name=f"hps{ch}")
                emit_sgroup(hoisted_ps, GROUPS[0][0], GROUPS[0][1],
                            slice((ch + 1) * CHW, (ch + 2) * CHW))
            while len(av_q) >= 2:
                av_q.pop(0)()

            # exp: alternate scalar-engine Exp and vector-engine bit trick,
            # both writing fp8e4 into the P ring
            slot = m0 % RING
            pdst = PT[:, slot * CHW:(slot + gsz) * CHW]
            if gi % 2 == 0:
                nc.scalar.activation(out=pdst, in_=ps[:, 0:gsz * CHW],
                                     func=Exp, bias=NLN2, scale=SCALE)
            else:
                nc.vector.tensor_scalar(
                    out=pdst.bitcast(I16), in0=ps[:, 0:gsz * CHW],
                    scalar1=A16, scalar2=B16, op0=mult, op1=add)

            def av_fn(po=po, m0=m0, gsz=gsz, ch=ch, ctr=ctr,
                      last=(gi == len(GROUPS) - 1)):
                tiles_done = m0 + gsz
                while ctr[0] < tiles_done:
                    m = ctr[0]
                    rs = m % RING
                    nc.tensor.matmul(
                        po[0:65, :],
                        lhsT=VT1[:, m, :],
                        rhs=PT[:, rs * CHW:(rs + 1) * CHW],
                        start=(m == 0), stop=(m == MT - 1),
                        skip_group_check=True)
                    ctr[0] += 1
                if last:
                    osb = esb.tile([65, CHW], F32, tag="osb", name=f"osb{ch}")
                    nc.vector.tensor_copy(out=osb, in_=po[0:65, :])
                    osbs[ch] = osb

            av_q.append(av_fn)
            if pending is not None:
                next(pending, None)
        if pending is not None:
            for _ in pending:
                pass
        if ch < NCH - 1:
            def pending_gen(ch=ch):
                while ch not in osbs:
                    yield  # wait for the lagged AV/osb of ch
                yield from epilogue_steps(ch)
            pending = pending_gen()
        else:
            pending = None
    while av_q:  # last AV groups + osb of the final chunk
        av_q.pop(0)()

    # final chunk's epilogue on rotating spool slots (attention done)
    osb = osbs.pop(NCH - 1)
    nsl = slice((NCH - 1) * CHW, NCH * CHW)
    pats = []
    for s4 in range(4):
        pat = spool.tile([128, GSZ * CHW], F32, tag="ps")
        nc.tensor.transpose(out=pat[:, 0:65],
                            in_=osb[:, s4 * 128:(s4 + 1) * 128],
                            identity=ID[0:65, 0:65])
        pats.append(pat)
    atn4 = esb.tile([128, 4 * C], BF16, tag="atn4")
    for s4 in range(4):
        rli = esb.tile([128, 1], F32, tag=f"rlif{s4}")
        nc.vector.reciprocal(out=rli, in_=pats[s4][:, 64:65])
        nc.scalar.activation(out=atn4[:, s4 * C:(s4 + 1) * C],
                             in_=pats[s4][:, 0:C], func=Copy,
                             bias=0.0, scale=rli)
    pp4 = spool.tile([128, GSZ * CHW], F32, tag="ps")
    nc.tensor.matmul(pp4[:, 0:4 * C], lhsT=WPB, rhs=atn4, start=True, stop=True)
    otb4 = esb.tile([128, 4 * C], BF16, tag="otb4")
    nc.scalar.activation(out=otb4, in_=pp4[:, 0:4 * C],
                         func=mybir.ActivationFunctionType.Identity,
                         bias=BP2, scale=1.0)
    for s4 in range(4):
        csl = slice((NCH - 1) * CHW + s4 * 128, (NCH - 1) * CHW + (s4 + 1) * 128)
        pf = spool.tile([128, GSZ * CHW], BF16, tag="ps")
        nc.tensor.transpose(out=pf[0:C, 0:128],
                            in_=otb4[:, s4 * C:(s4 + 1) * C], identity=IDB)
        nc.vector.tensor_tensor(out=OUT[:, csl], in0=pf[0:C, 0:128],
                                in1=X[:, csl], op=add)
        nc.sync.dma_start(out=y_d[:, csl], in_=OUT[:, csl])


def build_nc():
    nc = bacc.Bacc("TRN2", target_bir_lowering=False, debug=False)
    shapes = {
        "x": ([C, N], F32),
        "pf32": ([128, 1220], F32),
        "pb16": ([128, 640], BF16),
    }
    ins = {k: nc.dram_tensor(k, shp, dt, kind="ExternalInput").ap()
           for k, (shp, dt) in shapes.items()}
    y_d = nc.dram_tensor("y", [C, N], F32, kind="ExternalOutput").ap()
    with tile.TileContext(nc) as tc:
        with ExitStack() as ctx:
            attn_body(ctx, tc, ins, y_d)
    nc.compile()
    return nc


def host_params(inputs):
    """Build the packed parameter arrays shared by all cores."""
    import ml_dtypes
    f = lambda k: np.asarray(inputs[k], np.float32)

    def blockdiag(W):
        bd = np.zeros((128, 128), np.float32)
        bd[0:64, 0:64] = W.T
        bd[64:128, 64:128] = W.T
        return bd

    pf = np.zeros((128, 1220), np.float32)
    pf[:, 0:128] = np.eye(128, dtype=np.float32)
    pf[:, 128:640] = np.tile(f("bq"), 8)[None, :]
    pf[:, 640:1152] = np.tile(f("bk"), 8)[None, :]
    p2 = np.zeros((C, C), np.float32)
    for g in range(C // 2):
        p2[2 * g:2 * g + 2, 2 * g:2 * g + 2] = 0.5
    pf[0:C, 1152:1216] = p2
    pf[0:C, 1216] = f("gn_w")
    pf[0:C, 1217] = f("gn_b")
    pf[:, 1218] = np.tile(f("bv"), 2)
    pf[:, 1219] = np.tile(f("bp"), 2)

    pb = np.zeros((128, 640), np.float32)
    pb[:, 0:128] = blockdiag(f("Wq"))
    pb[:, 128:256] = blockdiag(f("Wk"))
    pb[:, 256:384] = blockdiag(f("Wv"))
    pb[:, 384:512] = blockdiag(f("Wp"))
    pb[:, 512:640] = np.eye(128, dtype=np.float32)
    return {"pf32": pf, "pb16": pb.astype(ml_dtypes.bfloat16)}


_NC_CACHE = {}


def get_nc():
    if "nc" not in _NC_CACHE:
        _NC_CACHE["nc"] = build_nc()
    return _NC_CACHE["nc"]


def make_in_maps(inputs):
    x = np.asarray(inputs["x"], np.float32)
    B = x.shape[0]
    p = host_params(inputs)
    return [dict(p, x=np.ascontiguousarray(x[b].reshape(C, N))) for b in range(B)]


def kernel(**inputs):
    from concourse.bass_utils import run_bass_kernel_spmd
    x = np.asarray(inputs["x"], np.float32)
    B = x.shape[0]
    nc = get_nc()
    in_maps = make_in_maps(inputs)
    res = run_bass_kernel_spmd(nc, in_maps, core_ids=list(range(B)))
    y = np.stack([res.results[b]["y"].reshape(C, 64, 64) for b in range(B)])
    return y.astype(np.float32)


# revision 34
# speedup vs baseline: 1.2704x; 1.2704x over previous
"""Trainium2 Bass kernel for nn_AttnBlock (B=8, C=64, H=W=64).

Data-parallel: 1 batch per NeuronCore (8 cores). Per core, full
flash-style attention over N=4096 positions with C=64 channels,
never materializing the (N, N) score tensor in HBM.

Per-core pipeline (all on-chip, x kept resident in SBUF):
  1. 8-way X DMA; GroupNorm via bn_stats + pair-combine matmul and a
     Quake-style rsqrt on the vector engine (avoids the sqrt act-table
     load; only the exp table set is ever loaded, via an early dummy).
  2. Transpose xn (c, n) -> XT (n, c) bf16 tiles via PE transposes;
     scalar-engine copies drain the transpose PSUM.
  3. Projections q/k (bf16, both array halves filled by an SBUF->SBUF
     dup DMA) and v in fp8e4 (scaled x64, folded back via Wp) with a
     ones column so row sums of exp(S) fall out of the AV matmul.
  4. Attention: S^T tiles via bf16 matmuls alternating PE halves.
     exp() is split across TWO engines: scalar-engine Exp -> fp8e4 and
     vector-engine Schraudolph bit-trick (s*a+b -> uint8, bitcast
     fp8e4), both writing a 12-slot fp8 P ring. AV matmuls run in fp8
     DoubleRow mode over m-tile pairs (2 contractions/cycle).
  5. Normalize/project/residual epilogue of chunk i interleaved into
     chunk i+1; PE pre-warmed with dummy matmuls so the HAM clock gate
     reaches 2.4 GHz before real work.

Self-contained: hardcodes all shapes; no file reads.
"""

import numpy as np
from contextlib import ExitStack

import concourse.bass as bass
import concourse.bacc as bacc
import concourse.tile as tile
from concourse import mybir

F32 = mybir.dt.float32
BF16 = mybir.dt.bfloat16
FP8 = mybir.dt.float8e4
U8 = mybir.dt.uint8
I16 = mybir.dt.int16
I32 = mybir.dt.int32

C = 64
N = 4096          # H*W
NCH = 8           # n-chunks of 512
CHW = 512         # chunk width
MT = 32           # m-tiles of 128
RING = 12         # P-ring slots
EPS = 1e-5
SCALE = 1.0 / 8.0  # C ** -0.5
A16 = 128.0 * 1.4426950408889634 * SCALE  # Schraudolph slope for bf16
B16 = 16128.0    # 127*128 exponent bias, shifted -ln2 (matches ACT bias)
QMAGIC = 0x5F3759DF

GSZ = 2
GROUPS = [(m0, min(GSZ, MT - m0)) for m0 in range(0, MT, GSZ)]

Exp = mybir.ActivationFunctionType.Exp
Copy = mybir.ActivationFunctionType.Copy
mult = mybir.AluOpType.mult
add = mybir.AluOpType.add
sub = mybir.AluOpType.subtract
b_xor = mybir.AluOpType.bitwise_xor
shr = mybir.AluOpType.arith_shift_right


def attn_body(ctx: ExitStack, tc: "tile.TileContext", ins: dict, y_d):
    nc = tc.nc

    persist = ctx.enter_context(tc.tile_pool(name="persist", bufs=1))
    sm = ctx.enter_context(tc.tile_pool(name="sm", bufs=2))

    # ---- persistent SBUF tiles ----
    X = persist.tile([C, N], F32, tag="X")
    XN = persist.tile([C, N], BF16, tag="XN")
    XT = persist.tile([128, 2048], BF16, tag="XT")      # ((h,w), c) tiles
    Q = persist.tile([128, N], BF16, tag="Q")
    K = persist.tile([128, N], BF16, tag="K")
    VT1 = persist.tile([128, MT, 65], BF16, tag="VT1")  # ((H,j), c | 1)
    PT = persist.tile([128, RING * CHW], BF16, tag="PT")  # exp(S^T) ring
    OUT = persist.tile([C, N], F32, tag="OUT")
    WS = persist.tile([128, CHW], BF16, tag="WS")       # warmup scratch

    PF = persist.tile([128, 1220], F32, tag="PF")
    PB = persist.tile([128, 640], BF16, tag="PB")
    ZB = persist.tile([128, 1], F32, tag="ZB")
    NLN2 = persist.tile([128, 1], F32, tag="NLN2")
    ID = PF[:, 0:128]          # fp32 identity (epilogue transposes)
    BQ2D = PF[:, 128:640]      # q bias along free axis, tiled x4
    BK2D = PF[:, 640:1152]
    P2 = PF[0:C, 1152:1216]    # pair-combine matrix
    GM = PF[0:C, 1216:1217]
    BT = PF[0:C, 1217:1218]
    BV2S = PF[:, 1218:1219]    # tile(bv,2)
    BP2 = PF[:, 1219:1220]
    PRJ = PB[:, 0:256]         # [Wq-blockdiag | Wk-blockdiag]
    WV = PB[:, 256:384]
    WPB = PB[:, 384:512]       # blockdiag(Wp)/VSCALE
    IDB = PB[:, 512:640]       # bf16 identity

    # ---- memsets + act-table prime + PE warmup + DMAs ----
    nc.vector.memset(ZB, 0.0)
    nc.vector.memset(NLN2, -0.6931471805599453)
    nc.vector.memset(WS, 0.0)
    nc.vector.memset(VT1[:, :, 64:65], 1.0)  # ones column -> row sums
    # dummy exp: pins the exp_and_others table load into setup dead time
    dume = sm.tile([C, 1], F32, tag="dume")
    nc.scalar.activation(out=dume, in_=ZB[0:C, :], func=Exp,
                         bias=ZB[0:C, :], scale=1.0)

    # X in 8 slices across the DMA-capable queues; params after
    engs = (nc.sync, nc.scalar, nc.gpsimd)
    for i in range(8):
        engs[i % 3].dma_start(out=X[:, i * 512:(i + 1) * 512],
                              in_=ins["x"][:, i * 512:(i + 1) * 512])
    nc.sync.dma_start(out=PF, in_=ins["pf32"])
    nc.scalar.dma_start(out=PB, in_=ins["pb16"])

    with tc.tile_pool(name="pst", space="PSUM", bufs=3) as pst, \
         tc.tile_pool(name="warm", space="PSUM", bufs=1) as warm:
        # ---- HAM warmup: ~18 back-to-back dummy matmuls (~4-7us busy) ----
        wps = warm.tile([128, CHW], F32, tag="wps")
        for _ in range(18):
            nc.tensor.matmul(wps, lhsT=WS[:, 0:128], rhs=WS,
                             start=True, stop=True)

        # ---- GroupNorm stats (chasing the DMAs) ----
        stats = sm.tile([C, 8, 6], F32, tag="stats")
        xg = X.rearrange("p (s f) -> p s f", s=8)
        for s in range(8):
            nc.vector.bn_stats(out=stats[:, s, :], in_=xg[:, s, :])
        mv = sm.tile([C, 2], F32, tag="mv")
        nc.vector.bn_aggr(out=mv, in_=stats)

        # st = [mean, E[x^2]] per channel
        st = sm.tile([C, 2], F32, tag="st")
        nc.vector.tensor_copy(out=st[:, 0:1], in_=mv[:, 0:1])
        msq = sm.tile([C, 1], F32, tag="msq")
        nc.vector.tensor_tensor(out=msq, in0=mv[:, 0:1], in1=mv[:, 0:1], op=mult)
        nc.vector.tensor_tensor(out=st[:, 1:2], in0=msq, in1=mv[:, 1:2], op=add)

        # group (pair) averages, broadcast back to both partitions
        mg_ps = pst.tile([128, 1024], F32, tag="pp")
        nc.tensor.matmul(mg_ps[0:C, 0:2], lhsT=P2, rhs=st, start=True, stop=True)
        for _ in range(10):  # keep PE warm across the GN scalar chain
            nc.tensor.matmul(wps, lhsT=WS[:, 0:128], rhs=WS,
                             start=True, stop=True)

        mu = sm.tile([C, 1], F32, tag="mu")
        nc.vector.tensor_copy(out=mu, in_=mg_ps[0:C, 0:1])
        musq = sm.tile([C, 1], F32, tag="musq")
        nc.vector.tensor_tensor(out=musq, in0=mu, in1=mu, op=mult)
        ve = sm.tile([C, 1], F32, tag="ve")
        nc.vector.tensor_tensor(out=ve, in0=mg_ps[0:C, 1:2], in1=musq, op=sub)
        nc.vector.tensor_scalar_add(out=ve, in0=ve, scalar1=EPS)

        # Quake rsqrt + 2 Newton iterations (all [64,1] DVE ops)
        ti = sm.tile([C, 1], I32, tag="ti")
        nc.vector.tensor_scalar(out=ti, in0=ve.bitcast(I32), scalar1=1,
                                scalar2=None, op0=shr)
        nc.vector.tensor_scalar(out=ti, in0=ti, scalar1=-1,
                                scalar2=None, op0=b_xor)
        nc.vector.tensor_scalar(out=ti, in0=ti, scalar1=QMAGIC + 1,
                                scalar2=None, op0=add)
        y0 = ti.bitcast(F32)
        rstd = sm.tile([C, 1], F32, tag="rstd")
        t_a = sm.tile([C, 1], F32, tag="t_a")
        for it in range(1):
            yin = y0 if it == 0 else rstd
            nc.vector.tensor_tensor(out=t_a, in0=yin, in1=yin, op=mult)
            nc.vector.tensor_tensor(out=t_a, in0=t_a, in1=ve, op=mult)
            nc.vector.tensor_scalar(out=t_a, in0=t_a, scalar1=-0.5,
                                    scalar2=1.5, op0=mult, op1=add)
            nc.vector.tensor_tensor(out=rstd, in0=yin, in1=t_a, op=mult)

        sc = sm.tile([C, 1], F32, tag="sc")
        nc.vector.tensor_tensor(out=sc, in0=rstd, in1=GM, op=mult)
        t3 = sm.tile([C, 1], F32, tag="t3")
        nc.vector.tensor_tensor(out=t3, in0=mu, in1=sc, op=mult)
        sh = sm.tile([C, 1], F32, tag="sh")
        nc.vector.tensor_tensor(out=sh, in0=BT, in1=t3, op=sub)

        # ---- per 1024-col block: normalize, transpose, project ----
        # Transposes run one block ahead of projections so every PE op's
        # input is ready ~a block early: the PE stream stays contiguous
        # through the setup phase and the HAM clock-gate reaches 2.4 GHz.
        def emit_projv(blk):
            for g in range(2):
                qk4 = pst.tile([128, 1024], F32, tag="pp", name=f"qk{blk}{g}")
                for t in range(4):
                    i = blk * 8 + g * 4 + t
                    nc.tensor.matmul(qk4[0:C, t * 256:(t + 1) * 256],
                                     lhsT=XT[:, i * C:(i + 1) * C],
                                     rhs=PRJ, start=True, stop=True)
                qk4r = qk4.rearrange("p (a b) -> p a b", a=4)
                hsl = slice(blk * 1024 + g * 512, blk * 1024 + (g + 1) * 512)
                q_out = Q[0:C, hsl].rearrange("p (a b) -> p a b", a=4)
                k_out = K[0:C, hsl].rearrange("p (a b) -> p a b", a=4)
                bq4 = BQ2D[0:C].rearrange("p (a b) -> p a b", a=4)
                bk4 = BK2D[0:C].rearrange("p (a b) -> p a b", a=4)
                nc.vector.tensor_tensor(out=q_out, in0=qk4r[0:C, :, 0:128],
                                        in1=bq4, op=add)
                nc.vector.tensor_tensor(out=k_out, in0=qk4r[0:C, :, 128:256],
                                        in1=bk4, op=add)
            # v projection: one 512-wide matmul covers 8 tiles
            vp8 = pst.tile([128, 1024], F32, tag="pp", name=f"v{blk}")
            nc.tensor.matmul(vp8[:, 0:512], lhsT=WV,
                             rhs=XT[:, blk * 512:(blk + 1) * 512],
                             start=True, stop=True)
            nc.scalar.activation(
                out=VT1[:, blk * 8:(blk + 1) * 8, 0:C],
                in_=vp8[:, 0:512].rearrange("p (a b) -> p a b", a=8),
                func=mybir.ActivationFunctionType.Identity,
                bias=BV2S, scale=1.0)
            bsl = slice(blk * 1024, (blk + 1) * 1024)
            # replicate q/k into PE-array upper half via off-engine DMA
            nc.sync.dma_start(out=Q[C:128, bsl], in_=Q[0:C, bsl])
            nc.scalar.dma_start(out=K[C:128, bsl], in_=K[0:C, bsl])

        prevb = None
        for blk in range(4):
            bsl = slice(blk * 1024, (blk + 1) * 1024)
            nc.gpsimd.tensor_scalar(out=XN[:, bsl], in0=X[:, bsl],
                                    scalar1=sc, scalar2=sh, op0=mult, op1=add)
            # transpose 8 x (64,128) -> (128,64); 4 per psum tile
            for g in range(2):
                tp4 = pst.tile([128, 1024], BF16, tag="pp", name=f"tp{blk}{g}")
                for t in range(4):
                    i = blk * 8 + g * 4 + t
                    nc.tensor.transpose(out=tp4[:, t * 64:(t + 1) * 64],
                                        in_=XN[:, i * 128:(i + 1) * 128],
                                        identity=IDB[0:C, 0:C])
                i0 = blk * 8 + g * 4
                nc.scalar.activation(out=XT[:, i0 * 64:(i0 + 4) * 64],
                                     in_=tp4[:, 0:256], func=Copy,
                                     bias=0.0, scale=1.0)
            if prevb is not None:
                emit_projv(prevb)
            prevb = blk
        emit_projv(prevb)

    # ---- attention ----
    spool = ctx.enter_context(tc.tile_pool(name="spool", space="PSUM", bufs=3))
    opool = ctx.enter_context(tc.tile_pool(name="opool", space="PSUM", bufs=1))
    epool = ctx.enter_context(tc.tile_pool(name="epool", space="PSUM", bufs=1))
    esb = ctx.enter_context(tc.tile_pool(name="esb", bufs=2))

    osbs = {}

    def epilogue_steps(ch):
        """Normalize by 1/l, project through Wp, add bias+residual, DMA out.

        Three dense PE batches (transposes | Wp matmuls | back-transposes)
        spaced one group apart; the elementwise chain between them runs on
        ACT/DVE well before the next PE batch needs it, so the in-order PE
        queue never blocks on a cross-engine dependency."""
        osb = osbs.pop(ch)
        nsl = slice(ch * CHW, (ch + 1) * CHW)
        pat4 = epool.tile([128, 260], F32, tag="ep", name=f"pa{ch}")
        for t in range(4):
            nc.tensor.transpose(out=pat4[:, t * 65:(t + 1) * 65],
                                in_=osb[:, t * 128:(t + 1) * 128],
                                identity=ID[0:65, 0:65])
        yield
        rli4 = esb.tile([128, 4], F32, tag="rli")
        nc.vector.reciprocal(
            out=rli4.rearrange("p (a b) -> p a b", b=1),
            in_=pat4.rearrange("p (a b) -> p a b", a=4)[:, :, 64:65])
        atn4 = esb.tile([128, 256], BF16, tag="atn")
        for t in range(4):
            nc.scalar.activation(out=atn4[:, t * C:(t + 1) * C],
                                 in_=pat4[:, t * 65:t * 65 + C],
                                 func=Copy, bias=0.0, scale=rli4[:, t:t + 1])
        yield
        pp4 = epool.tile([128, 256], F32, tag="ep", name=f"pp{ch}")
        nc.tensor.matmul(pp4, lhsT=WPB, rhs=atn4, start=True, stop=True)
        yield
        otb4 = esb.tile([128, 256], BF16, tag="otb")
        nc.scalar.activation(out=otb4, in_=pp4,
                             func=mybir.ActivationFunctionType.Identity,
                             bias=BP2, scale=1.0)
        yield
        pf4 = epool.tile([C, 512], BF16, tag="ep", name=f"pf{ch}")
        for t in range(4):
            nc.tensor.transpose(out=pf4[:, t * 128:(t + 1) * 128],
                                in_=otb4[:, t * C:(t + 1) * C], identity=IDB)
        yield
        for h in range(2):
            csl = slice(ch * CHW + h * 256, ch * CHW + (h + 1) * 256)
            nc.vector.tensor_tensor(out=OUT[:, csl],
                                    in0=pf4[:, h * 256:(h + 1) * 256],
                                    in1=X[:, csl], op=add)
        nc.sync.dma_start(out=y_d[:, nsl], in_=OUT[:, nsl])

    pending = None
    av_q = []  # AV closures lag their exp by TWO groups: keeps the AV
               # matmuls (which wait on exp output) off the critical
               # exp -> S(psum-slot) -> exp dependency cycle

    def emit_sgroup(ps, m0, gsz, nsl):
        for t in range(gsz):
            m = m0 + t
            h = (m % 2) * C  # alternate the two 64-row PE halves
            nc.tensor.matmul(
                ps[:, t * CHW:(t + 1) * CHW],
                lhsT=K[h:h + C, m * 128:(m + 1) * 128],
                rhs=Q[h:h + C, nsl],
                start=True, stop=True, tile_position=(h, 0))

    hoisted_ps = None
    for ch in range(NCH):
        nsl = slice(ch * CHW, (ch + 1) * CHW)
        po = opool.tile([128, CHW], F32, tag="po")
        ctr = [0]  # per-chunk AV pair counter (closures capture their own)
        for gi, (m0, gsz) in enumerate(GROUPS):
            if gi == 0 and hoisted_ps is not None:
                ps = hoisted_ps
                hoisted_ps = None
            else:
                ps = spool.tile([128, GSZ * CHW], F32, tag="ps")
                emit_sgroup(ps, m0, gsz, nsl)
            if gi == len(GROUPS) - 1 and ch < NCH - 1:
                # hoist next chunk's first S-group ahead of trailing AV work
                hoisted_ps = spool.tile([128, GSZ * CHW], F32, tag="ps",
                                        name=f"hps{ch}")
                emit_sgroup(hoisted_ps, GROUPS[0][0], GROUPS[0][1],
                            slice((ch + 1) * CHW, (ch + 2) * CHW))
            while len(av_q) >= 2:
                av_q.pop(0)()

            # exp: alternate scalar-engine Exp and vector-engine bit trick,
            # both writing fp8e4 into the P ring
            slot = m0 % RING
            pdst = PT[:, slot * CHW:(slot + gsz) * CHW]
            if gi % 2 == 0:
                nc.scalar.activation(out=pdst, in_=ps[:, 0:gsz * CHW],
                                     func=Exp, bias=NLN2, scale=SCALE)
            else:
                nc.vector.tensor_scalar(
                    out=pdst.bitcast(I16), in0=ps[:, 0:gsz * CHW],
                    scalar1=A16, scalar2=B16, op0=mult, op1=add)

            def av_fn(po=po, m0=m0, gsz=gsz, ch=ch, ctr=ctr,
                      last=(gi == len(GROUPS) - 1)):
                tiles_done = m0 + gsz
                while ctr[0] < tiles_done:
                    m = ctr[0]
                    rs = m % RING
                    nc.tensor.matmul(
                        po[0:65, :],
                        lhsT=VT1[:, m, :],
                        rhs=PT[:, rs * CHW:(rs + 1) * CHW],
                        start=(m == 0), stop=(m == MT - 1),
                        skip_group_check=True)
                    ctr[0] += 1
                if last:
                    osb = esb.tile([65, CHW], F32, tag="osb", name=f"osb{ch}")
                    nc.vector.tensor_copy(out=osb, in_=po[0:65, :])
                    osbs[ch] = osb

            av_q.append(av_fn)
            if pending is not None:
                next(pending, None)
        if pending is not None:
            for _ in pending:
                pass
        if ch < NCH - 1:
            def pending_gen(ch=ch):
                while ch not in osbs:
                    yield  # wait for the lagged AV/osb of ch
                yield from epilogue_steps(ch)
            pending = pending_gen()
        else:
            pending = None
    while av_q:  # last AV groups + osb of the final chunk
        av_q.pop(0)()

    # final chunk's epilogue on rotating spool slots (attention done)
    osb = osbs.pop(NCH - 1)
    nsl = slice((NCH - 1) * CHW, NCH * CHW)
    pats = []
    for s4 in range(4):
        pat = spool.tile([128, GSZ * CHW], F32, tag="ps")
        nc.tensor.transpose(out=pat[:, 0:65],
                            in_=osb[:, s4 * 128:(s4 + 1) * 128],
                            identity=ID[0:65, 0:65])
        pats.append(pat)
    atn4 = esb.tile([128, 4 * C], BF16, tag="atn4")
    for s4 in range(4):
        rli = esb.tile([128, 1], F32, tag=f"rlif{s4}")
        nc.vector.reciprocal(out=rli, in_=pats[s4][:, 64:65])
        nc.scalar.activation(out=atn4[:, s4 * C:(s4 + 1) * C],
                             in_=pats[s4][:, 0:C], func=Copy,
                             bias=0.0, scale=rli)
    pp4 = spool.tile([128, GSZ * CHW], F32, tag="ps")
    nc.tensor.matmul(pp4[:, 0:4 * C], lhsT=WPB, rhs=atn4, start=True, stop=True)
    otb4 = esb.tile([128, 4 * C], BF16, tag="otb4")
    nc.scalar.activation(out=otb4, in_=pp4[:, 0:4 * C],
                         func=mybir.ActivationFunctionType.Identity,
                         bias=BP2, scale=1.0)
    for s4 in range(4):
        csl = slice((NCH - 1) * CHW + s4 * 128, (NCH - 1) * CHW + (s4 + 1) * 128)
        pf = spool.tile([128, GSZ * CHW], BF16, tag="ps")
        nc.tensor.transpose(out=pf[0:C, 0:128],
                            in_=otb4[:, s4 * C:(s4 + 1) * C], identity=IDB)
        nc.vector.tensor_tensor(out=OUT[:, csl], in0=pf[0:C, 0:128],
                                in1=X[:, csl], op=add)
        nc.sync.dma_start(out=y_d[:, csl], in_=OUT[:, csl])


def build_nc():
    nc = bacc.Bacc("TRN2", target_bir_lowering=False, debug=False)
    shapes = {
        "x": ([C, N], F32),
        "pf32": ([128, 1220], F32),
        "pb16": ([128, 640], BF16),
    }
    ins = {k: nc.dram_tensor(k, shp, dt, kind="ExternalInput").ap()
           for k, (shp, dt) in shapes.items()}
    y_d = nc.dram_tensor("y", [C, N], F32, kind="ExternalOutput").ap()
    with tile.TileContext(nc) as tc:
        with ExitStack() as ctx:
            attn_body(ctx, tc, ins, y_d)
    nc.compile()
    return nc


def host_params(inputs):
    """Build the packed parameter arrays shared by all cores."""
    import ml_dtypes
    f = lambda k: np.asarray(inputs[k], np.float32)

    def blockdiag(W):
        bd = np.zeros((128, 128), np.float32)
        bd[0:64, 0:64] = W.T
        bd[64:128, 64:128] = W.T
        return bd

    pf = np.zeros((128, 1220), np.float32)
    pf[:, 0:128] = np.eye(128, dtype=np.float32)
    pf[:, 128:640] = np.tile(f("bq"), 8)[None, :]
    pf[:, 640:1152] = np.tile(f("bk"), 8)[None, :]
    p2 = np.zeros((C, C), np.float32)
    for g in range(C // 2):
        p2[2 * g:2 * g + 2, 2 * g:2 * g + 2] = 0.5
    pf[0:C, 1152:1216] = p2
    pf[0:C, 1216] = f("gn_w")
    pf[0:C, 1217] = f("gn_b")
    pf[:, 1218] = np.tile(f("bv"), 2)
    pf[:, 1219] = np.tile(f("bp"), 2)

    pb = np.zeros((128, 640), np.float32)
    pb[:, 0:128] = blockdiag(f("Wq"))
    pb[:, 128:256] = blockdiag(f("Wk"))
    pb[:, 256:384] = blockdiag(f("Wv"))
    pb[:, 384:512] = blockdiag(f("Wp"))
    pb[:, 512:640] = np.eye(128, dtype=np.float32)
    return {"pf32": pf, "pb16": pb.astype(ml_dtypes.bfloat16)}


_NC_CACHE = {}


def get_nc():
    if "nc" not in _NC_CACHE:
        _NC_CACHE["nc"] = build_nc()
    return _NC_CACHE["nc"]


def make_in_maps(inputs):
    x = np.asarray(inputs["x"], np.float32)
    B = x.shape[0]
    p = host_params(inputs)
    return [dict(p, x=np.ascontiguousarray(x[b].reshape(C, N))) for b in range(B)]


def kernel(**inputs):
    from concourse.bass_utils import run_bass_kernel_spmd
    x = np.asarray(inputs["x"], np.float32)
    B = x.shape[0]
    nc = get_nc()
    in_maps = make_in_maps(inputs)
    res = run_bass_kernel_spmd(nc, in_maps, core_ids=list(range(B)))
    y = np.stack([res.results[b]["y"].reshape(C, 64, 64) for b in range(B)])
    return y.astype(np.float32)
